# revision 1
# baseline (speedup 1.0000x reference)
"""Trainium2 Bass kernel for nn_Attention_35021163332119.

Full multi-head attention: qkv = x @ w_qkv; RoPE(q, k); softmax(q k^T / sqrt(dh)) v;
out = heads @ w_out + b_out.  B=2, N=2048, DIM=1024, H=16, DH=64.

Sharding: 8 cores = (batch b in {0,1}) x (head-group g in {0..3} of 4 heads).
Each core computes its 4 heads end-to-end plus the partial output projection
for its head-group's rows of w_out; the host sums the 4 partials per batch
and adds b_out.

On-core layout: x is host-transposed to xT [DIM, N] so the contraction dim
sits on SBUF partitions.  q,k are produced transposed ([dh, n], head pairs
stacked on 128 partitions) straight out of the QKV matmul; v is produced in
natural [n, dh] layout with an extra ones column, so the PV matmul (M=65)
also accumulates the softmax denominator in row 64.  RoPE's interleaved
pair-rotation is a 128x128 +/-1 permutation matmul on the PE plus two DVE
multiplies against cos/sin tables.

Precision: QKV / RoPE / scores matmuls in float32r (full PE rate, ~1e-4
rel err).  The probability-side (exp output, v, output projection) runs in
bf16 — softmax weights and the final linear tolerate it.  Scores matmul
pairs are emitted A,B,A,B across the two stacked heads so the K=64 matmuls
run concurrently in disjoint PE row groups.
"""

import numpy as np

B, N, DIM, H, DH = 2, 2048, 1024, 16, 64
ROPE_BASE = 10000.0
SCALE = DH ** -0.5
N_CORES = 8
G = 4                 # heads per core
KT = DIM // 128       # contraction tiles
NT = N // 128         # sequence tiles
NCH = N // 512        # 512-wide moving chunks

_cache = {}


def _rope_tables():
    inv_freq = (1.0 / (ROPE_BASE ** (np.arange(0, DH, 2, dtype=np.float32) / DH)))
    t = np.arange(N, dtype=np.float32)
    freqs = t[:, None] * inv_freq[None, :]          # [N, DH/2]
    freqs = np.repeat(freqs, 2, axis=-1)            # [N, DH] interleaved
    cosT = np.cos(freqs).T.astype(np.float32)       # [DH, N]
    sinT = np.sin(freqs).T.astype(np.float32)
    cos2 = np.concatenate([cosT, cosT], axis=0)     # [128, N] two heads stacked
    sin2 = np.concatenate([sinT, sinT], axis=0)
    return np.ascontiguousarray(cos2), np.ascontiguousarray(sin2)


def _p2t():
    # rot = P2 @ qT with P2 = blockdiag(P, P), P[2t, 2t+1] = -1, P[2t+1, 2t] = 1
    # matmul computes lhsT.T @ rhs, so pass P2.T
    p = np.zeros((DH, DH), dtype=np.float32)
    for t in range(DH // 2):
        p[2 * t, 2 * t + 1] = -1.0
        p[2 * t + 1, 2 * t] = 1.0
    p2 = np.zeros((128, 128), dtype=np.float32)
    p2[:DH, :DH] = p
    p2[DH:, DH:] = p
    return np.ascontiguousarray(p2.T)


def _build():
    if "nc" in _cache:
        return _cache["nc"]

    import concourse.mybir as mybir
    import concourse.tile as tile
    from concourse import bacc

    F32 = mybir.dt.float32
    F32R = mybir.dt.float32r
    BF16 = mybir.dt.bfloat16
    EXP = mybir.ActivationFunctionType.Exp

    nc = bacc.Bacc("TRN2", target_bir_lowering=False, debug=False)
    xT_d = nc.dram_tensor("xT", [DIM, N], BF16, kind="ExternalInput")
    wqk_d = nc.dram_tensor("wqk", [DIM, 4 * 128], BF16, kind="ExternalInput")
    wv_d = nc.dram_tensor("wv", [DIM, G * DH], BF16, kind="ExternalInput")
    wout_d = nc.dram_tensor("wout", [G * DH, DIM], BF16, kind="ExternalInput")
    cos_d = nc.dram_tensor("cos2", [128, N], BF16, kind="ExternalInput")
    sin_d = nc.dram_tensor("sin2", [128, N], BF16, kind="ExternalInput")
    p2t_d = nc.dram_tensor("p2t", [128, 128], F32, kind="ExternalInput")
    part_d = nc.dram_tensor("part", [N, DIM], F32, kind="ExternalOutput")

    with tile.TileContext(nc) as tc:
        with tc.tile_pool(name="persist", bufs=1) as persist, \
             tc.tile_pool(name="att", bufs=5) as att, \
             tc.tile_pool(name="norm_w", bufs=2) as norm_w, \
             tc.tile_pool(name="outp", bufs=3) as outp, \
             tc.tile_pool(name="xph", bufs=1) as xph, \
             tc.tile_pool(name="rope_w", bufs=1) as rope_w, \
             tc.tile_pool(name="ps", bufs=3, space="PSUM") as ps, \
             tc.tile_pool(name="pso", bufs=2, space="PSUM") as pso:

            # ---- persistent tiles ----
            qk_sb = [persist.tile([128, N], F32R, tag=f"qk{m}", name=f"qk{m}")
                     for m in range(4)]          # q01T, q23T, k01T, k23T
            v_aug = persist.tile([128, NT, G, DH + 1], BF16, tag="vaug")
            wout_sb = [persist.tile([128, DIM], BF16, tag=f"wo{kk}", name=f"wo{kk}")
                       for kk in range(2)]
            outT = [persist.tile([128, N], BF16, tag=f"outT{p}", name=f"outT{p}")
                    for p in range(2)]

            # ---- phase-1 tiles ----
            xT = [xph.tile([128, N], BF16, tag=f"xT{k}", name=f"xT{k}")
                  for k in range(KT)]
            wqk = [xph.tile([128, 4 * 128], BF16, tag=f"wqk{k}", name=f"wqk{k}")
                   for k in range(KT)]
            wv = [xph.tile([128, G * DH], BF16, tag=f"wv{k}", name=f"wv{k}")
                  for k in range(KT)]
            cos2 = xph.tile([128, N], BF16, tag="cos2")
            sin2 = xph.tile([128, N], BF16, tag="sin2")
            p2t = xph.tile([128, 128], F32R, tag="p2t")
            ones_col = xph.tile([128, NT, G, 1], F32, tag="ones")

            for kk in range(2):
                nc.sync.dma_start(
                    out=wout_sb[kk],
                    in_=wout_d.ap().rearrange("(t p) m -> t p m", p=128)[kk])
            nc.sync.dma_start(out=cos2, in_=cos_d.ap())
            nc.sync.dma_start(out=sin2, in_=sin_d.ap())
            nc.sync.dma_start(out=p2t, in_=p2t_d.ap().bitcast(F32R))
            nc.vector.memset(ones_col, 1.0)
            for k in range(KT):
                for q in range(2):
                    qsl = slice(q * (N // 2), (q + 1) * (N // 2))
                    nc.sync.dma_start(
                        out=xT[k][:, qsl],
                        in_=xT_d.ap().rearrange(
                            "(t p) n -> t p n", p=128)[k][:, qsl])
                nc.sync.dma_start(
                    out=wqk[k],
                    in_=wqk_d.ap().rearrange("(t p) m -> t p m", p=128)[k])
                nc.sync.dma_start(
                    out=wv[k],
                    in_=wv_d.ap().rearrange("(t p) m -> t p m", p=128)[k])

            def qk_pair(p):
                # q pair tile (m=p) and k pair tile (m=2+p); two 512-chunks
                # per PSUM tile so the PSUM->SBUF copies are [128, 1024]
                for m in (p, 2 + p):
                    for c2 in range(2):
                        mm_ps = ps.tile([128, 1024], F32, tag="s", name="mm_qk")
                        for half in range(2):
                            hsl = slice(half * 512, (half + 1) * 512)
                            csl = slice(c2 * 1024 + half * 512,
                                        c2 * 1024 + (half + 1) * 512)
                            for k in range(KT):
                                nc.tensor.matmul(
                                    mm_ps[:, hsl],
                                    wqk[k][:, m * 128:(m + 1) * 128],
                                    xT[k][:, csl],
                                    start=(k == 0), stop=(k == KT - 1))
                        nc.scalar.copy(
                            qk_sb[m][:, c2 * 1024:(c2 + 1) * 1024], mm_ps)

            def rope_pair(p):
                for m in (p, 2 + p):
                    tmp = rope_w.tile([128, N], F32R, tag="ropetmp")
                    for c2 in range(2):
                        rot_ps = ps.tile([128, 1024], F32, tag="s",
                                         name="mm_rot")
                        for half in range(2):
                            csl = slice(c2 * 1024 + half * 512,
                                        c2 * 1024 + (half + 1) * 512)
                            nc.tensor.matmul(
                                rot_ps[:, half * 512:(half + 1) * 512],
                                p2t, qk_sb[m][:, csl],
                                start=True, stop=True)
                        nc.vector.tensor_mul(
                            tmp[:, c2 * 1024:(c2 + 1) * 1024], rot_ps,
                            sin2[:, c2 * 1024:(c2 + 1) * 1024])
                    nc.vector.tensor_mul(qk_sb[m], qk_sb[m], cos2)
                    nc.vector.tensor_add(qk_sb[m], qk_sb[m], tmp)

            def v_all():
                for tn in range(NT):
                    mm_ps = ps.tile([128, 1024], F32, tag="s", name="mm_v")
                    for k in range(KT):
                        nc.tensor.matmul(
                            mm_ps[:, 0:G * DH],
                            xT[k][:, tn * 128:(tn + 1) * 128],
                            wv[k],
                            start=(k == 0), stop=(k == KT - 1))
                    nc.vector.tensor_copy(
                        v_aug[:, tn, :, 0:DH],
                        mm_ps[:, 0:G * DH].rearrange("p (h d) -> p h d", h=G))
                nc.vector.tensor_copy(v_aug[:, :, :, DH:DH + 1], ones_col)

            def attention(p, iq):
                """One (head-pair, i-quarter of 512) block.  Scores PSUM
                tiles hold two j-tiles x 512 i-columns per head, so each exp
                op still covers 1024 elements while the PV accumulators only
                need one PSUM bank per head (leaving 3 scores buffers for
                pipeline slack)."""
                qT = qk_sb[p]
                kTt = qk_sb[2 + p]
                i0 = iq * 512
                isl = slice(i0, i0 + 512)
                o_ps = [pso.tile([DH + 1, 512], F32, tag="o", name=f"o{hh}")
                        for hh in range(2)]

                def emit_pv(jj, exps):
                    for hh in range(2):
                        for half in range(2):
                            j = 2 * jj + half
                            nc.tensor.matmul(
                                o_ps[hh],
                                v_aug[:, j, 2 * p + hh, :],
                                exps[hh][:, half * 512:(half + 1) * 512],
                                start=(j == 0), stop=(j == NT - 1))

                pend = None   # software pipeline: PV of jj-1 runs while exp
                for jj in range(NT // 2):   # of jj occupies the scalar engine
                    s_ps = [ps.tile([128, 1024], F32, tag="s", name=f"s{hh}")
                            for hh in range(2)]
                    # scores: interleave heads A,B,A,B -> disjoint PE row
                    # groups run concurrently
                    for half in range(2):
                        j = 2 * jj + half
                        jsl = slice(j * 128, (j + 1) * 128)
                        for hh in range(2):
                            hsl = slice(hh * DH, (hh + 1) * DH)
                            nc.tensor.matmul(
                                s_ps[hh][:, half * 512:(half + 1) * 512],
                                kTt[hsl, jsl], qT[hsl, isl],
                                start=True, stop=True)
                    exps = []
                    for hh in range(2):
                        expT = att.tile([128, 1024], BF16, tag="exp")
                        nc.scalar.activation(expT, s_ps[hh], EXP, scale=SCALE)
                        exps.append(expT)
                    if pend is not None:
                        emit_pv(jj - 1, pend)
                    pend = exps
                emit_pv(NT // 2 - 1, pend)
                # move PV accumulators to SBUF so PSUM frees immediately;
                # normalization happens off the critical path
                for hh in range(2):
                    o_sb = norm_w.tile([DH + 1, 512], F32, tag=f"osb{hh}",
                                       name=f"osb{hh}")
                    nc.vector.tensor_copy(o_sb, o_ps[hh])
                    recip0 = norm_w.tile([1, 512], F32, tag=f"r0{hh}",
                                         name=f"r0{hh}")
                    nc.sync.dma_start(out=recip0, in_=o_sb[DH:DH + 1, :])
                    nc.vector.reciprocal_approx_fast(recip0, recip0)
                    bc = norm_w.tile([DH, 512], F32, tag=f"bc{hh}",
                                     name=f"bc{hh}")
                    nc.gpsimd.partition_broadcast(bc, recip0)
                    if hh == 0:
                        nc.vector.tensor_mul(outT[p][0:DH, isl],
                                             o_sb[0:DH, :], bc)
                    else:
                        tmpb = norm_w.tile([DH, 512], BF16, tag="tmpb")
                        nc.vector.tensor_mul(tmpb, o_sb[0:DH, :], bc)
                        nc.sync.dma_start(out=outT[p][DH:2 * DH, isl],
                                          in_=tmpb)

            def proj_tile(tn):
                nsl = slice(tn * 128, (tn + 1) * 128)
                f_ps = ps.tile([128, 1024], F32, tag="s", name="f_ps")
                for c2 in range(2):
                    c2sl = slice(c2 * 512, (c2 + 1) * 512)
                    for kk in range(2):
                        nc.tensor.matmul(
                            f_ps[:, c2sl],
                            outT[kk][:, nsl], wout_sb[kk][:, c2sl],
                            start=(kk == 0), stop=(kk == 1))
                out_sb = outp.tile([128, DIM], F32, tag="osb")
                if tn % 2 == 0:
                    nc.scalar.copy(out_sb, f_ps)
                else:
                    nc.vector.tensor_copy(out_sb, f_ps)
                nc.sync.dma_start(
                    out=part_d.ap().rearrange("(t p) m -> t p m", p=128)[tn],
                    in_=out_sb)

            # ---- emission order ----
            qk_pair(0)
            rope_pair(0)
            qk_pair(1)
            rope_pair(1)
            v_all()
            for iq in range(4):
                attention(0, iq)
            for iq in range(3):
                attention(1, iq)
            # projection tiles 0-11 only need i-quarters 0-2 of both pairs;
            # emit them before the last attention block so they fill the
            # tail instead of serializing after it
            for tn in range(12):
                proj_tile(tn)
            attention(1, 3)
            for tn in range(12, NT):
                proj_tile(tn)
    nc.compile()
    _cache["nc"] = nc
    return nc


def kernel(x, w_qkv, w_out, b_out, _trace=False):
    import ml_dtypes
    from concourse.bass_utils import run_bass_kernel_spmd

    x = np.asarray(x, dtype=np.float32)
    w_qkv = np.asarray(w_qkv, dtype=np.float32)
    w_out = np.asarray(w_out, dtype=np.float32)
    b_out = np.asarray(b_out, dtype=np.float32)

    cos2, sin2 = _rope_tables()
    p2t = _p2t()

    in_maps = []
    for c in range(N_CORES):
        b, g = divmod(c, G)
        cols = []
        for blk in range(2):                      # q block, k block
            base = blk * H * DH + g * G * DH
            cols.append(w_qkv[:, base:base + G * DH])
        wqk_c = np.ascontiguousarray(np.concatenate(cols, axis=1))  # [DIM, 512]
        wv_c = np.ascontiguousarray(
            w_qkv[:, 2 * H * DH + g * G * DH: 2 * H * DH + (g + 1) * G * DH])
        wout_c = np.ascontiguousarray(
            w_out[g * G * DH:(g + 1) * G * DH, :]).astype(ml_dtypes.bfloat16)
        in_maps.append({
            "xT": np.ascontiguousarray(x[b].T).astype(ml_dtypes.bfloat16),
            "wqk": wqk_c.astype(ml_dtypes.bfloat16),
            "wv": wv_c.astype(ml_dtypes.bfloat16),
            "wout": wout_c,
            "cos2": cos2.astype(ml_dtypes.bfloat16),
            "sin2": sin2.astype(ml_dtypes.bfloat16),
            "p2t": p2t,
        })

    nc = _build()
    res = run_bass_kernel_spmd(nc, in_maps, core_ids=list(range(N_CORES)),
                               trace=_trace)
    out = np.empty((B, N, DIM), dtype=np.float32)
    for b in range(B):
        acc = res.results[G * b]["part"].copy()
        for g in range(1, G):
            acc += res.results[G * b + g]["part"]
        out[b] = acc + b_out
    if _trace:
        kernel.last_results = res
    return out



# revision 3
# speedup vs baseline: 1.0712x; 1.0712x over previous
"""Trainium2 Bass kernel for nn_Attention_35021163332119.

Full multi-head attention: qkv = x @ w_qkv; RoPE(q, k); softmax(q k^T / sqrt(dh)) v;
out = heads @ w_out + b_out.  B=2, N=2048, DIM=1024, H=16, DH=64.

Sharding: 8 cores = (batch b in {0,1}) x (head-group g in {0..3} of 4 heads).
Each core computes its 4 heads end-to-end plus the partial output projection
for its head-group's rows of w_out; the host sums the 4 partials per batch
and adds b_out.

On-core layout: x is host-transposed to xT [DIM, N] so the contraction dim
sits on SBUF partitions.  q,k are produced transposed ([dh, n], head pairs
stacked on 128 partitions) straight out of the QKV matmul; v is produced in
natural [n, dh] layout with an extra ones column, so the PV matmul (M=65)
also accumulates the softmax denominator in row 64.  RoPE's interleaved
pair-rotation is a 128x128 +/-1 permutation matmul on the PE plus two DVE
multiplies against cos/sin tables.

All matmuls run in bf16 (full PE stream rate; fp32r streams at half rate on
HW).  Scores accumulate fp32 in PSUM; exp runs on the Scalar engine reading
PSUM directly.

Schedule: the attention phase is Scalar(exp)-bound (~2.3us/jj vs ~1.7us/jj
of PE work), so only K (both pairs), V tiles 0-7 and the first Q chunk are
computed up front; the remaining Q chunks, V tiles 8-15 and the output
projection are emitted as small "filler" pieces inside the attention
blocks' jj loops where the PE has slack.  exp starts ~35us into the kernel
instead of ~94us.
"""

import numpy as np

B, N, DIM, H, DH = 2, 2048, 1024, 16, 64
ROPE_BASE = 10000.0
SCALE = DH ** -0.5
N_CORES = 8
G = 4                 # heads per core
KT = DIM // 128       # contraction tiles
NT = N // 128         # sequence tiles

_cache = {}


def _rope_tables():
    inv_freq = (1.0 / (ROPE_BASE ** (np.arange(0, DH, 2, dtype=np.float32) / DH)))
    t = np.arange(N, dtype=np.float32)
    freqs = t[:, None] * inv_freq[None, :]          # [N, DH/2]
    freqs = np.repeat(freqs, 2, axis=-1)            # [N, DH] interleaved
    cosT = np.cos(freqs).T.astype(np.float32)       # [DH, N]
    sinT = np.sin(freqs).T.astype(np.float32)
    cos2 = np.concatenate([cosT, cosT], axis=0)     # [128, N] two heads stacked
    sin2 = np.concatenate([sinT, sinT], axis=0)
    return np.ascontiguousarray(cos2), np.ascontiguousarray(sin2)


def _p2t():
    # rot = P2 @ qT with P2 = blockdiag(P, P), P[2t, 2t+1] = -1, P[2t+1, 2t] = 1
    # matmul computes lhsT.T @ rhs, so pass P2.T
    p = np.zeros((DH, DH), dtype=np.float32)
    for t in range(DH // 2):
        p[2 * t, 2 * t + 1] = -1.0
        p[2 * t + 1, 2 * t] = 1.0
    p2 = np.zeros((128, 128), dtype=np.float32)
    p2[:DH, :DH] = p
    p2[DH:, DH:] = p
    return np.ascontiguousarray(p2.T)


def _build():
    if "nc" in _cache:
        return _cache["nc"]

    import concourse.mybir as mybir
    import concourse.tile as tile
    from concourse import bacc

    F32 = mybir.dt.float32
    BF16 = mybir.dt.bfloat16
    EXP = mybir.ActivationFunctionType.Exp

    nc = bacc.Bacc("TRN2", target_bir_lowering=False, debug=False)
    xT_d = nc.dram_tensor("xT", [DIM, N], BF16, kind="ExternalInput")
    wqk_d = nc.dram_tensor("wqk", [DIM, 4 * 128], BF16, kind="ExternalInput")
    wv_d = nc.dram_tensor("wv", [DIM, G * DH], BF16, kind="ExternalInput")
    wout_d = nc.dram_tensor("wout", [G * DH, DIM], BF16, kind="ExternalInput")
    cos_d = nc.dram_tensor("cos2", [128, N], BF16, kind="ExternalInput")
    sin_d = nc.dram_tensor("sin2", [128, N], BF16, kind="ExternalInput")
    p2t_d = nc.dram_tensor("p2t", [128, 128], BF16, kind="ExternalInput")
    part_d = nc.dram_tensor("part", [N, DIM], F32, kind="ExternalOutput")

    with tile.TileContext(nc) as tc:
        with tc.tile_pool(name="persist", bufs=1) as persist, \
             tc.tile_pool(name="att", bufs=5) as att, \
             tc.tile_pool(name="norm_w", bufs=2) as norm_w, \
             tc.tile_pool(name="outp", bufs=3) as outp, \
             tc.tile_pool(name="xph", bufs=1) as xph, \
             tc.tile_pool(name="rope_w", bufs=2) as rope_w, \
             tc.tile_pool(name="ps", bufs=3, space="PSUM") as ps, \
             tc.tile_pool(name="pso", bufs=2, space="PSUM") as pso:

            # ---- persistent tiles ----
            qk_sb = [persist.tile([128, N], BF16, tag=f"qk{m}", name=f"qk{m}")
                     for m in range(4)]          # q01T, q23T, k01T, k23T
            v_aug = [persist.tile([128, G, DH + 1], BF16, tag=f"vaug{tn}",
                                  name=f"vaug{tn}")
                     for tn in range(NT)]        # per-j-tile for precise deps
            wout_sb = [persist.tile([128, DIM], BF16, tag=f"wo{kk}", name=f"wo{kk}")
                       for kk in range(2)]
            outT = [[persist.tile([128, 512], BF16, tag=f"outT{p}_{iq}",
                                  name=f"outT{p}_{iq}")
                     for iq in range(4)] for p in range(2)]

            # ---- phase-1 tiles ----
            xT = [xph.tile([128, N], BF16, tag=f"xT{k}", name=f"xT{k}")
                  for k in range(KT)]
            wqk = [xph.tile([128, 4 * 128], BF16, tag=f"wqk{k}", name=f"wqk{k}")
                   for k in range(KT)]
            wv = [xph.tile([128, G * DH], BF16, tag=f"wv{k}", name=f"wv{k}")
                  for k in range(KT)]
            cos2 = xph.tile([128, N], BF16, tag="cos2")
            sin2 = xph.tile([128, N], BF16, tag="sin2")
            p2t = xph.tile([128, 128], BF16, tag="p2t")

            # ---- input DMA, priority order ----
            # k chains need wqk + xT cols 0:1024 first; v needs wv; rope
            # needs cos/sin/p2t by ~15us; wout not until the projection.
            for k in range(KT):
                nc.sync.dma_start(
                    out=wqk[k],
                    in_=wqk_d.ap().rearrange("(t p) m -> t p m", p=128)[k])
                nc.sync.dma_start(
                    out=xT[k][:, 0:1024],
                    in_=xT_d.ap().rearrange(
                        "(t p) n -> t p n", p=128)[k][:, 0:1024])
            for k in range(KT):
                nc.sync.dma_start(
                    out=wv[k],
                    in_=wv_d.ap().rearrange("(t p) m -> t p m", p=128)[k])
            nc.sync.dma_start(out=cos2, in_=cos_d.ap())
            nc.sync.dma_start(out=sin2, in_=sin_d.ap())
            nc.sync.dma_start(out=p2t, in_=p2t_d.ap())
            for k in range(KT):
                nc.sync.dma_start(
                    out=xT[k][:, 1024:2048],
                    in_=xT_d.ap().rearrange(
                        "(t p) n -> t p n", p=128)[k][:, 1024:2048])
            for kk in range(2):
                nc.sync.dma_start(
                    out=wout_sb[kk],
                    in_=wout_d.ap().rearrange("(t p) m -> t p m", p=128)[kk])
            for tn in range(NT):
                nc.vector.memset(v_aug[tn][:, :, DH:DH + 1], 1.0)

            # ---- emitters ----
            def qk_chunk_mm(m, c2, half, klo, khi, holder):
                """Piece of the [128,1024] QKV chain for tile m, chunk c2:
                k-range [klo,khi) of the `half` 512-accumulation."""
                if holder.get("t") is None:
                    holder["t"] = ps.tile([128, 1024], F32, tag="s",
                                          name=f"mm_qk{m}_{c2}")
                mm_ps = holder["t"]
                hsl = slice(half * 512, (half + 1) * 512)
                csl = slice(c2 * 1024 + half * 512, c2 * 1024 + (half + 1) * 512)
                for k in range(klo, khi):
                    nc.tensor.matmul(
                        mm_ps[:, hsl],
                        wqk[k][:, m * 128:(m + 1) * 128],
                        xT[k][:, csl],
                        start=(k == 0), stop=(k == KT - 1))

            def qk_chunk_finish(m, c2, holder, use_vector):
                csl = slice(c2 * 1024, (c2 + 1) * 1024)
                if use_vector:
                    nc.vector.tensor_copy(qk_sb[m][:, csl], holder["t"])
                else:
                    nc.scalar.copy(qk_sb[m][:, csl], holder["t"])
                holder["t"] = None

            def rope_rot(m, c2, holder):
                """rot = P2 @ qk chunk -> PSUM."""
                holder["t"] = ps.tile([128, 1024], F32, tag="s",
                                      name=f"mm_rot{m}_{c2}")
                for half in range(2):
                    csl = slice(c2 * 1024 + half * 512,
                                c2 * 1024 + (half + 1) * 512)
                    nc.tensor.matmul(
                        holder["t"][:, half * 512:(half + 1) * 512],
                        p2t, qk_sb[m][:, csl],
                        start=True, stop=True)

            def rope_finish(m, c2, holder):
                csl = slice(c2 * 1024, (c2 + 1) * 1024)
                tmp = rope_w.tile([128, 1024], BF16, tag="ropetmp")
                nc.vector.tensor_mul(tmp, holder["t"], sin2[:, csl])
                nc.vector.tensor_mul(qk_sb[m][:, csl], qk_sb[m][:, csl],
                                     cos2[:, csl])
                nc.vector.tensor_add(qk_sb[m][:, csl], qk_sb[m][:, csl], tmp)
                holder["t"] = None

            def qk_chunk_full(m, c2, use_vector=False):
                h = {}
                for half in range(2):
                    qk_chunk_mm(m, c2, half, 0, KT, h)
                qk_chunk_finish(m, c2, h, use_vector)
                rope_rot(m, c2, h)
                rope_finish(m, c2, h)

            def v_tile(tn):
                mm_ps = ps.tile([128, 1024], F32, tag="s", name=f"mm_v{tn}")
                for k in range(KT):
                    nc.tensor.matmul(
                        mm_ps[:, 0:G * DH],
                        xT[k][:, tn * 128:(tn + 1) * 128],
                        wv[k],
                        start=(k == 0), stop=(k == KT - 1))
                nc.vector.tensor_copy(
                    v_aug[tn][:, :, 0:DH],
                    mm_ps[:, 0:G * DH].rearrange("p (h d) -> p h d", h=G))

            def proj_tile(tn, copy_eng):
                nsl = slice((tn % 4) * 128, (tn % 4) * 128 + 128)
                iq = tn // 4
                f_ps = ps.tile([128, 1024], F32, tag="s", name=f"f_ps{tn}")
                for c2 in range(2):
                    c2sl = slice(c2 * 512, (c2 + 1) * 512)
                    for kk in range(2):
                        nc.tensor.matmul(
                            f_ps[:, c2sl],
                            outT[kk][iq][:, nsl], wout_sb[kk][:, c2sl],
                            start=(kk == 0), stop=(kk == 1))
                out_sb = outp.tile([128, DIM], F32, tag="osb")
                copy_eng(out_sb, f_ps)
                nc.sync.dma_start(
                    out=part_d.ap().rearrange("(t p) m -> t p m", p=128)[tn],
                    in_=out_sb)

            def attention(p, iq, fillers=None):
                """One (head-pair, i-quarter of 512) block.  Per j-tile a
                single PSUM tile holds BOTH heads' scores side by side
                ([j=128, head0 i | head1 i]) so one ACTIVATE covers both
                heads and only one ps buffer is consumed per j-step - the
                scores pipeline keeps a 2-step cushion even when a filler
                chain occupies a third buffer.  `fillers` is a list of
                zero-arg closures emitting small PE pieces into the PE
                slack (the block is Scalar-bound); one is drained every
                second j-step."""
                fillers = list(fillers or [])
                qT = qk_sb[p]
                kTt = qk_sb[2 + p]
                i0 = iq * 512
                isl = slice(i0, i0 + 512)
                o_ps = [pso.tile([DH + 1, 512], F32, tag="o", name=f"o{hh}")
                        for hh in range(2)]

                def emit_pv(j, expT):
                    for hh in range(2):
                        nc.tensor.matmul(
                            o_ps[hh],
                            v_aug[j][:, 2 * p + hh, :],
                            expT[:, hh * 512:(hh + 1) * 512],
                            start=(j == 0), stop=(j == NT - 1))

                pend = None   # software pipeline: PV of j-1 runs while exp
                for j in range(NT):  # of j occupies the scalar engine
                    s_ps = ps.tile([128, 1024], F32, tag="s", name=f"s{j}")
                    jsl = slice(j * 128, (j + 1) * 128)
                    # heads A,B -> disjoint PE row groups run concurrently
                    for hh in range(2):
                        hsl = slice(hh * DH, (hh + 1) * DH)
                        nc.tensor.matmul(
                            s_ps[:, hh * 512:(hh + 1) * 512],
                            kTt[hsl, jsl], qT[hsl, isl],
                            start=True, stop=True)
                    expT = att.tile([128, 1024], BF16, tag="exp")
                    nc.scalar.activation(expT, s_ps, EXP, scale=SCALE)
                    if pend is not None:
                        emit_pv(j - 1, pend)
                    if fillers and j % 2 == 1:
                        fillers.pop(0)()
                    pend = expT
                emit_pv(NT - 1, pend)
                while fillers:
                    fillers.pop(0)()
                # move PV accumulators to SBUF so PSUM frees immediately;
                # normalization happens off the critical path
                for hh in range(2):
                    o_sb = norm_w.tile([DH + 1, 512], F32, tag=f"osb{hh}",
                                       name=f"osb{hh}")
                    nc.vector.tensor_copy(o_sb, o_ps[hh])
                    recip0 = norm_w.tile([1, 512], F32, tag=f"r0{hh}",
                                         name=f"r0{hh}")
                    nc.sync.dma_start(out=recip0, in_=o_sb[DH:DH + 1, :])
                    nc.vector.reciprocal_approx_fast(recip0, recip0)
                    bc = norm_w.tile([DH, 512], F32, tag=f"bc{hh}",
                                     name=f"bc{hh}")
                    nc.gpsimd.partition_broadcast(bc, recip0)
                    if hh == 0:
                        nc.vector.tensor_mul(outT[p][iq][0:DH, :],
                                             o_sb[0:DH, :], bc)
                    else:
                        tmpb = norm_w.tile([DH, 512], BF16, tag="tmpb")
                        nc.vector.tensor_mul(tmpb, o_sb[0:DH, :], bc)
                        nc.sync.dma_start(out=outT[p][iq][DH:2 * DH, :],
                                          in_=tmpb)

            def q_chunk_fillers(m, c2):
                """Spread one q chunk (16 matmuls + copy + rope) over a
                block's 8 filler slots."""
                h = {}
                f = []
                for half in range(2):
                    for klo in (0, 3, 6):
                        khi = min(klo + 3, KT)
                        f.append(lambda m=m, c2=c2, half=half, klo=klo,
                                 khi=khi: qk_chunk_mm(m, c2, half, klo, khi, h))
                f.append(lambda: (qk_chunk_finish(m, c2, h, True),
                                  rope_rot(m, c2, h)))
                f.append(lambda: rope_finish(m, c2, h))
                return f

            # ---- emission order ----
            # upfront: k for both pairs (roped), v tiles 0-7, q pair0 chunk0
            for c2 in range(2):
                for p in range(2):
                    qk_chunk_full(2 + p, c2)
            for tn in range(8):
                v_tile(tn)
            qk_chunk_full(0, 0)

            # attention blocks with filler work in the PE slack
            attention(0, 0, [lambda tn=tn: v_tile(tn) for tn in range(8, NT)])
            attention(0, 1, q_chunk_fillers(0, 1))
            attention(0, 2, q_chunk_fillers(1, 0))
            attention(0, 3, q_chunk_fillers(1, 1))
            attention(1, 0)
            attention(1, 1, [lambda tn=tn: proj_tile(tn, nc.vector.tensor_copy)
                             for tn in range(0, 4)])
            attention(1, 2, [lambda tn=tn: proj_tile(tn, nc.vector.tensor_copy)
                             for tn in range(4, 8)])
            attention(1, 3, [lambda tn=tn: proj_tile(tn, nc.vector.tensor_copy)
                             for tn in range(8, 12)])
            for tn in range(12, NT):
                proj_tile(tn, nc.scalar.copy if tn % 2 else
                          nc.vector.tensor_copy)
    nc.compile()
    _cache["nc"] = nc
    return nc


def kernel(x, w_qkv, w_out, b_out, _trace=False):
    import ml_dtypes
    from concourse.bass_utils import run_bass_kernel_spmd

    x = np.asarray(x, dtype=np.float32)
    w_qkv = np.asarray(w_qkv, dtype=np.float32)
    w_out = np.asarray(w_out, dtype=np.float32)
    b_out = np.asarray(b_out, dtype=np.float32)

    cos2, sin2 = _rope_tables()
    p2t = _p2t()

    in_maps = []
    for c in range(N_CORES):
        b, g = divmod(c, G)
        cols = []
        for blk in range(2):                      # q block, k block
            base = blk * H * DH + g * G * DH
            cols.append(w_qkv[:, base:base + G * DH])
        wqk_c = np.ascontiguousarray(np.concatenate(cols, axis=1))  # [DIM, 512]
        wv_c = np.ascontiguousarray(
            w_qkv[:, 2 * H * DH + g * G * DH: 2 * H * DH + (g + 1) * G * DH])
        wout_c = np.ascontiguousarray(
            w_out[g * G * DH:(g + 1) * G * DH, :]).astype(ml_dtypes.bfloat16)
        in_maps.append({
            "xT": np.ascontiguousarray(x[b].T).astype(ml_dtypes.bfloat16),
            "wqk": wqk_c.astype(ml_dtypes.bfloat16),
            "wv": wv_c.astype(ml_dtypes.bfloat16),
            "wout": wout_c,
            "cos2": cos2.astype(ml_dtypes.bfloat16),
            "sin2": sin2.astype(ml_dtypes.bfloat16),
            "p2t": p2t.astype(ml_dtypes.bfloat16),
        })

    nc = _build()
    res = run_bass_kernel_spmd(nc, in_maps, core_ids=list(range(N_CORES)),
                               trace=_trace)
    out = np.empty((B, N, DIM), dtype=np.float32)
    for b in range(B):
        acc = res.results[G * b]["part"].copy()
        for g in range(1, G):
            acc += res.results[G * b + g]["part"]
        out[b] = acc + b_out
    if _trace:
        kernel.last_results = res
    return out


# revision 12
# speedup vs baseline: 1.1544x; 1.0777x over previous
"""Trainium2 Bass kernel for nn_Attention_35021163332119.

Full multi-head attention: qkv = x @ w_qkv; RoPE(q, k); softmax(q k^T / sqrt(dh)) v;
out = heads @ w_out + b_out.  B=2, N=2048, DIM=1024, H=16, DH=64.

Sharding: 8 cores = (batch b in {0,1}) x (head-group g in {0..3} of 4 heads).
Each core computes its 4 heads end-to-end plus the partial output projection
for its head-group's rows of w_out; the host sums the 4 partials per batch
and adds b_out.

On-core layout: x is host-transposed to xT [DIM, N] so the contraction dim
sits on SBUF partitions.  q,k are produced transposed ([dh, n], head pairs
stacked on 128 partitions) straight out of the QKV matmul; v is produced in
natural [n, dh] layout with an extra ones column, so the PV matmul (M=65)
also accumulates the softmax denominator in row 64.  RoPE's interleaved
pair-rotation is a 128x128 +/-1 permutation matmul on the PE plus two DVE
multiplies against cos/sin tables.

All matmuls run in bf16 (full PE stream rate; fp32r streams at half rate on
HW).  Scores accumulate fp32 in PSUM; exp runs on the Scalar engine reading
PSUM directly.  Per j-tile a single PSUM tile holds BOTH heads' scores side
by side so one ACTIVATE covers both heads and only one ps buffer is
consumed per j-step.

Schedule: the attention phase is Scalar(exp)-bound (~1.16us/j-step vs
~0.65us of PE work), so only K (both pairs), V tiles 0-7 and the first Q
chunk are computed up front; the remaining Q chunks, V tiles 8-15 and the
output projection are emitted as small "filler" pieces inside the attention
blocks' j loops where the PE has slack.  Inputs are loaded with a handful
of batched DMAs (multi-tile interleaved layout) because each DMA dispatch
costs ~600ns on the sync engine.
"""

import numpy as np

B, N, DIM, H, DH = 2, 2048, 1024, 16, 64
ROPE_BASE = 10000.0
SCALE = DH ** -0.5
N_CORES = 8
G = 4                 # heads per core
KT = DIM // 128       # contraction tiles
NT = N // 128         # sequence tiles

_cache = {}


def _rope_tables():
    inv_freq = (1.0 / (ROPE_BASE ** (np.arange(0, DH, 2, dtype=np.float32) / DH)))
    t = np.arange(N, dtype=np.float32)
    freqs = t[:, None] * inv_freq[None, :]          # [N, DH/2]
    freqs = np.repeat(freqs, 2, axis=-1)            # [N, DH] interleaved
    cosT = np.cos(freqs).T.astype(np.float32)       # [DH, N]
    sinT = np.sin(freqs).T.astype(np.float32)
    cos2 = np.concatenate([cosT, cosT], axis=0)     # [128, N] two heads stacked
    sin2 = np.concatenate([sinT, sinT], axis=0)
    return np.ascontiguousarray(cos2), np.ascontiguousarray(sin2)


def _p2t():
    # rot = P2 @ qT with P2 = blockdiag(P, P), P[2t, 2t+1] = -1, P[2t+1, 2t] = 1
    # matmul computes lhsT.T @ rhs, so pass P2.T
    p = np.zeros((DH, DH), dtype=np.float32)
    for t in range(DH // 2):
        p[2 * t, 2 * t + 1] = -1.0
        p[2 * t + 1, 2 * t] = 1.0
    p2 = np.zeros((128, 128), dtype=np.float32)
    p2[:DH, :DH] = p
    p2[DH:, DH:] = p
    return np.ascontiguousarray(p2.T)


def _build():
    if "nc" in _cache:
        return _cache["nc"]

    import concourse.mybir as mybir
    import concourse.tile as tile
    from concourse import bacc

    F32 = mybir.dt.float32
    F32R = mybir.dt.float32r
    BF16 = mybir.dt.bfloat16
    EXP = mybir.ActivationFunctionType.Exp

    nc = bacc.Bacc("TRN2", target_bir_lowering=False, debug=False)
    xT_d = nc.dram_tensor("xT", [DIM, N], BF16, kind="ExternalInput")
    wqk_d = nc.dram_tensor("wqk", [DIM, 4 * 128], BF16, kind="ExternalInput")
    wv_d = nc.dram_tensor("wv", [DIM, G * DH], BF16, kind="ExternalInput")
    wout_d = nc.dram_tensor("wout", [G * DH, DIM], BF16, kind="ExternalInput")
    cos_d = nc.dram_tensor("cos2", [128, N], BF16, kind="ExternalInput")
    sin_d = nc.dram_tensor("sin2", [128, N], BF16, kind="ExternalInput")
    p2t_d = nc.dram_tensor("p2t", [128, 128], BF16, kind="ExternalInput")
    ones_d = nc.dram_tensor("onesr", [DH + 1, DH], mybir.dt.float32,
                            kind="ExternalInput")
    part_d = nc.dram_tensor("part", [N, DIM], BF16, kind="ExternalOutput")

    with tile.TileContext(nc) as tc:
        with tc.tile_pool(name="persist", bufs=1) as persist, \
             tc.tile_pool(name="att", bufs=5) as att, \
             tc.tile_pool(name="norm_w", bufs=2) as norm_w, \
             tc.tile_pool(name="outp", bufs=3) as outp, \
             tc.tile_pool(name="xph", bufs=1) as xph, \
             tc.tile_pool(name="rope_w", bufs=2) as rope_w, \
             tc.tile_pool(name="ps", bufs=3, space="PSUM") as ps, \
             tc.tile_pool(name="pso", bufs=2, space="PSUM") as pso:

            # ---- persistent tiles ----
            qk_sb = [persist.tile([128, N], BF16, tag=f"qk{m}", name=f"qk{m}")
                     for m in range(4)]          # q01T, q23T, k01T, k23T
            v_aug = [persist.tile([128, G, DH + 1], BF16, tag=f"vaug{tn}",
                                  name=f"vaug{tn}")
                     for tn in range(NT)]        # per-j-tile for precise deps
            wout_sb = persist.tile([128, 2, DIM], BF16, tag="wo", name="wo")
            outT = [[persist.tile([128, 512], BF16, tag=f"outT{p}_{iq}",
                                  name=f"outT{p}_{iq}")
                     for iq in range(4)] for p in range(2)]

            # ---- phase-1 tiles (multi-tile interleaved so one DMA loads all
            # k-tiles of a column chunk) ----
            xT = xph.tile([128, KT, N], BF16, tag="xT", name="xT")
            wqk = xph.tile([128, KT, 4 * 128], BF16, tag="wqk", name="wqk")
            wv = xph.tile([128, KT, G * DH], BF16, tag="wv", name="wv")
            cos2 = xph.tile([128, N], BF16, tag="cos2")
            sin2 = xph.tile([128, N], BF16, tag="sin2")
            p2t = xph.tile([128, 128], BF16, tag="p2t")
            ones_r = xph.tile([DH + 1, DH], F32R, tag="ones_r")

            # ---- input DMA, priority order, batched ----
            xT_r = xT_d.ap().rearrange("(t p) n -> p t n", p=128)
            wqk_r = wqk_d.ap().rearrange("(t p) m -> p t m", p=128)
            nc.sync.dma_start(out=wqk[:, :, 256:512], in_=wqk_r[:, :, 256:512])
            nc.sync.dma_start(out=xT[:, :, 0:512], in_=xT_r[:, :, 0:512])
            nc.sync.dma_start(out=xT[:, :, 512:1024], in_=xT_r[:, :, 512:1024])
            nc.sync.dma_start(
                out=wv, in_=wv_d.ap().rearrange("(t p) m -> p t m", p=128))
            nc.sync.dma_start(out=cos2, in_=cos_d.ap())
            nc.sync.dma_start(out=sin2, in_=sin_d.ap())
            nc.sync.dma_start(out=p2t, in_=p2t_d.ap())
            nc.sync.dma_start(out=ones_r, in_=ones_d.ap().bitcast(F32R))
            nc.sync.dma_start(out=wqk[:, :, 0:256], in_=wqk_r[:, :, 0:256])
            nc.sync.dma_start(out=xT[:, :, 1024:1536], in_=xT_r[:, :, 1024:1536])
            nc.sync.dma_start(out=xT[:, :, 1536:2048], in_=xT_r[:, :, 1536:2048])
            nc.sync.dma_start(
                out=wout_sb, in_=wout_d.ap().rearrange("(t p) m -> p t m", p=128))
            for tn in range(NT):
                nc.vector.memset(v_aug[tn][:, :, DH:DH + 1], 1.0)

            # ---- emitters ----
            def qk_chunk_mm(m, c2, half, klo, khi, holder):
                """Piece of the [128,1024] QKV chain for tile m, chunk c2:
                k-range [klo,khi) of the `half` 512-accumulation."""
                if holder.get("t") is None:
                    holder["t"] = ps.tile([128, 1024], F32, tag="s",
                                          name=f"mm_qk{m}_{c2}")
                mm_ps = holder["t"]
                hsl = slice(half * 512, (half + 1) * 512)
                csl = slice(c2 * 1024 + half * 512, c2 * 1024 + (half + 1) * 512)
                for k in range(klo, khi):
                    nc.tensor.matmul(
                        mm_ps[:, hsl],
                        wqk[:, k, m * 128:(m + 1) * 128],
                        xT[:, k, csl],
                        start=(k == 0), stop=(k == KT - 1))

            def qk_chunk_finish(m, c2, holder, use_vector):
                csl = slice(c2 * 1024, (c2 + 1) * 1024)
                if use_vector:
                    nc.vector.tensor_copy(qk_sb[m][:, csl], holder["t"])
                else:
                    nc.scalar.copy(qk_sb[m][:, csl], holder["t"])
                holder["t"] = None

            def rope_rot(m, c2, holder):
                """rot = P2 @ qk chunk -> PSUM."""
                holder["t"] = ps.tile([128, 1024], F32, tag="s",
                                      name=f"mm_rot{m}_{c2}")
                for half in range(2):
                    csl = slice(c2 * 1024 + half * 512,
                                c2 * 1024 + (half + 1) * 512)
                    nc.tensor.matmul(
                        holder["t"][:, half * 512:(half + 1) * 512],
                        p2t, qk_sb[m][:, csl],
                        start=True, stop=True)

            def rope_finish(m, c2, holder):
                csl = slice(c2 * 1024, (c2 + 1) * 1024)
                tmp = rope_w.tile([128, 1024], BF16, tag="ropetmp")
                nc.vector.tensor_mul(tmp, holder["t"], sin2[:, csl])
                nc.vector.tensor_mul(qk_sb[m][:, csl], qk_sb[m][:, csl],
                                     cos2[:, csl])
                nc.vector.tensor_add(qk_sb[m][:, csl], qk_sb[m][:, csl], tmp)
                holder["t"] = None

            def v_tile(tn):
                mm_ps = ps.tile([128, 1024], F32, tag="s", name=f"mm_v{tn}")
                for k in range(KT):
                    nc.tensor.matmul(
                        mm_ps[:, 0:G * DH],
                        xT[:, k, tn * 128:(tn + 1) * 128],
                        wv[:, k, :],
                        start=(k == 0), stop=(k == KT - 1))
                nc.vector.tensor_copy(
                    v_aug[tn][:, :, 0:DH],
                    mm_ps[:, 0:G * DH].rearrange("p (h d) -> p h d", h=G))

            def proj_tile(tn, copy_eng):
                nsl = slice((tn % 4) * 128, (tn % 4) * 128 + 128)
                iq = tn // 4
                f_ps = ps.tile([128, 1024], F32, tag="s", name=f"f_ps{tn}")
                for c2 in range(2):
                    c2sl = slice(c2 * 512, (c2 + 1) * 512)
                    for kk in range(2):
                        nc.tensor.matmul(
                            f_ps[:, c2sl],
                            outT[kk][iq][:, nsl], wout_sb[:, kk, c2sl],
                            start=(kk == 0), stop=(kk == 1))
                out_sb = outp.tile([128, DIM], BF16, tag="osb")
                copy_eng(out_sb, f_ps)
                nc.sync.dma_start(
                    out=part_d.ap().rearrange("(t p) m -> t p m", p=128)[tn],
                    in_=out_sb)

            def attention(p, iq, fillers=None, start_j=1, pre=None):
                """One (head-pair, i-quarter of 512) block.  Per j-tile a
                single PSUM tile holds BOTH heads' scores side by side
                ([j=128, head0 i | head1 i]) so one ACTIVATE covers both
                heads and only one ps buffer is consumed per j-step - the
                scores pipeline keeps a 2-step cushion even when a filler
                chain occupies a third buffer.  `fillers` is a list of
                zero-arg closures emitting small PE pieces into the PE
                slack (the block is Scalar-bound); one is drained every
                second j-step starting at `start_j` (delay it if the filler
                depends on the previous block's normalization).  `pre` is
                the previous block's norm tail, emitted at j==1 so the PE
                does not stall on it at the block boundary.  Returns this
                block's norm tail closure."""
                fillers = list(fillers or [])
                qT = qk_sb[p]
                kTt = qk_sb[2 + p]
                i0 = iq * 512
                isl = slice(i0, i0 + 512)
                o_ps = [pso.tile([DH + 1, 512], F32, tag="o", name=f"o{hh}")
                        for hh in range(2)]

                def emit_pv(j, expT):
                    for hh in range(2):
                        nc.tensor.matmul(
                            o_ps[hh],
                            v_aug[j][:, 2 * p + hh, :],
                            expT[:, hh * 512:(hh + 1) * 512],
                            start=(j == 0), stop=(j == NT - 1))

                pend = None   # software pipeline: PV of j-1 runs while exp
                for j in range(NT):  # of j occupies the scalar engine
                    s_ps = ps.tile([128, 1024], F32, tag="s", name=f"s{j}")
                    jsl = slice(j * 128, (j + 1) * 128)
                    # heads A,B -> disjoint PE row groups run concurrently
                    for hh in range(2):
                        hsl = slice(hh * DH, (hh + 1) * DH)
                        nc.tensor.matmul(
                            s_ps[:, hh * 512:(hh + 1) * 512],
                            kTt[hsl, jsl], qT[hsl, isl],
                            start=True, stop=True)
                    expT = att.tile([128, 1024], BF16, tag="exp")
                    nc.scalar.activation(expT, s_ps, EXP, scale=SCALE)
                    if pend is not None:
                        emit_pv(j - 1, pend)
                    if j == 1 and pre is not None:
                        pre()
                    if fillers and j >= start_j and (j - start_j) % 2 == 0:
                        fillers.pop(0)()
                    pend = expT
                emit_pv(NT - 1, pend)
                while fillers:
                    fillers.pop(0)()
                # copy the PV accumulators to SBUF now (frees PSUM for the
                # next block); denominator broadcast + reciprocal + multiply
                # are deferred into the next block via the returned closure
                o_sbs = []
                for hh in range(2):
                    o_sb = norm_w.tile([DH + 1, 512], F32R, tag=f"osb{hh}",
                                       name=f"osb{hh}")
                    nc.vector.tensor_copy(o_sb, o_ps[hh])
                    o_sbs.append(o_sb)

                def norm_tail():
                    # broadcast the denominator row 64 across partitions
                    # 0..63 with a K=1 ones matmul (fp32r, exact for a x1.0
                    # multiply), then reciprocal+multiply on the DVE
                    bc_ps = ps.tile([128, 1024], F32, tag="s", name="bc_ps")
                    for hh in range(2):
                        nc.tensor.matmul(
                            bc_ps[0:DH, hh * 512:(hh + 1) * 512],
                            ones_r[DH:DH + 1, :],
                            o_sbs[hh][DH:DH + 1, :],
                            start=True, stop=True)
                    for hh in range(2):
                        bc = norm_w.tile([DH, 512], F32, tag=f"bc{hh}",
                                         name=f"bc{hh}")
                        nc.vector.reciprocal_approx_fast(
                            bc, bc_ps[0:DH, hh * 512:(hh + 1) * 512])
                        if hh == 0:
                            nc.vector.tensor_mul(outT[p][iq][0:DH, :],
                                                 o_sbs[hh][0:DH, :], bc)
                        else:
                            tmpb = norm_w.tile([DH, 512], BF16, tag="tmpb")
                            nc.vector.tensor_mul(tmpb, o_sbs[hh][0:DH, :], bc)
                            nc.sync.dma_start(out=outT[p][iq][DH:2 * DH, :],
                                              in_=tmpb)
                return norm_tail

            def q_chunk_fillers(m, c2):
                """Spread one q chunk (16 matmuls + copy + rope) over a
                block's 8 filler slots."""
                h = {}
                f = []
                for half in range(2):
                    for klo in (0, 3, 6):
                        khi = min(klo + 3, KT)
                        f.append(lambda m=m, c2=c2, half=half, klo=klo,
                                 khi=khi: qk_chunk_mm(m, c2, half, klo, khi, h))
                f.append(lambda: (qk_chunk_finish(m, c2, h, True),
                                  rope_rot(m, c2, h)))
                f.append(lambda: rope_finish(m, c2, h))
                return f

            # ---- emission order ----
            # upfront: k for both pairs (roped), v tiles 0-7, q pair0 chunk0.
            # chains before rots so the PE never waits on the PSUM->SBUF copy
            hold = {}
            for c2 in range(2):
                for p in range(2):
                    h = hold[(p, c2)] = {}
                    for half in range(2):
                        qk_chunk_mm(2 + p, c2, half, 0, KT, h)
                    qk_chunk_finish(2 + p, c2, h, False)
                if c2 == 0:
                    for tn in range(9):
                        v_tile(tn)
                for p in range(2):
                    rope_rot(2 + p, c2, hold[(p, c2)])
                    rope_finish(2 + p, c2, hold[(p, c2)])
            h = {}
            for half in range(2):
                qk_chunk_mm(0, 0, half, 0, KT, h)
            qk_chunk_finish(0, 0, h, False)
            rope_rot(0, 0, h)
            rope_finish(0, 0, h)

            # attention blocks with filler work in the PE slack
            nt = attention(0, 0, [lambda tn=tn: v_tile(tn)
                                  for tn in range(9, NT)])
            nt = attention(0, 1, q_chunk_fillers(0, 1), start_j=2, pre=nt)
            nt = attention(0, 2, q_chunk_fillers(1, 0), start_j=2, pre=nt)
            nt = attention(0, 3, q_chunk_fillers(1, 1), start_j=2, pre=nt)
            nt = attention(1, 0, pre=nt)
            nt = attention(1, 1, [lambda tn=tn: proj_tile(
                tn, nc.vector.tensor_copy) for tn in range(0, 4)],
                start_j=7, pre=nt)
            nt = attention(1, 2, [lambda tn=tn: proj_tile(
                tn, nc.vector.tensor_copy) for tn in range(4, 8)],
                start_j=7, pre=nt)
            nt = attention(1, 3, [lambda tn=tn: proj_tile(
                tn, nc.vector.tensor_copy) for tn in range(8, 12)],
                start_j=7, pre=nt)
            nt()
            for tn in range(12, NT):
                proj_tile(tn, nc.vector.tensor_copy if tn % 2 else
                          nc.scalar.copy)
    nc.compile()
    _cache["nc"] = nc
    return nc


def kernel(x, w_qkv, w_out, b_out, _trace=False):
    import ml_dtypes
    from concourse.bass_utils import run_bass_kernel_spmd

    x = np.asarray(x, dtype=np.float32)
    w_qkv = np.asarray(w_qkv, dtype=np.float32)
    w_out = np.asarray(w_out, dtype=np.float32)
    b_out = np.asarray(b_out, dtype=np.float32)

    cos2, sin2 = _rope_tables()
    p2t = _p2t()

    in_maps = []
    for c in range(N_CORES):
        b, g = divmod(c, G)
        cols = []
        for blk in range(2):                      # q block, k block
            base = blk * H * DH + g * G * DH
            cols.append(w_qkv[:, base:base + G * DH])
        wqk_c = np.ascontiguousarray(np.concatenate(cols, axis=1))  # [DIM, 512]
        wv_c = np.ascontiguousarray(
            w_qkv[:, 2 * H * DH + g * G * DH: 2 * H * DH + (g + 1) * G * DH])
        wout_c = np.ascontiguousarray(
            w_out[g * G * DH:(g + 1) * G * DH, :]).astype(ml_dtypes.bfloat16)
        in_maps.append({
            "xT": np.ascontiguousarray(x[b].T).astype(ml_dtypes.bfloat16),
            "wqk": wqk_c.astype(ml_dtypes.bfloat16),
            "wv": wv_c.astype(ml_dtypes.bfloat16),
            "wout": wout_c,
            "cos2": cos2.astype(ml_dtypes.bfloat16),
            "sin2": sin2.astype(ml_dtypes.bfloat16),
            "p2t": p2t.astype(ml_dtypes.bfloat16),
            "onesr": np.ones((DH + 1, DH), dtype=np.float32),
        })

    nc = _build()
    res = run_bass_kernel_spmd(nc, in_maps, core_ids=list(range(N_CORES)),
                               trace=_trace)
    out = np.empty((B, N, DIM), dtype=np.float32)
    for b in range(B):
        acc = res.results[G * b]["part"].astype(np.float32)
        for g in range(1, G):
            acc += res.results[G * b + g]["part"].astype(np.float32)
        out[b] = acc + b_out
    if _trace:
        kernel.last_results = res
    return out
